# revision 3
# baseline (speedup 1.0000x reference)
"""Trainium2 Bass kernel: 16-head attention (RoPE + T5 rel bias, causal) on 8 cores.

Sharding: tensor-parallel over heads (2 heads/core, both batch elements on
every core). Each core computes q/k/v^T for its heads from a replicated
pre-transposed x, runs causal flash attention, AllGathers ctx^T, and produces
its 256 columns of the final dense projection. Host concatenates columns.
"""

import math
import sys

sys.path.insert(0, "/opt/trn_rl_repo")

import numpy as np
import ml_dtypes

import concourse.bass as bass
import concourse.mybir as mybir
import concourse.tile as tile
from concourse import bacc
from concourse import bass_utils

BF16 = np.float16

B, S, H = 2, 2048, 2048
NT = B * S            # 4096 tokens
N_HEADS = 16
HD = 128              # head dim
NC = 8                # cores
HPC = N_HEADS // NC   # heads per core = 2
KH = H // 128         # 16 contraction tiles over hidden
NUM_BUCKETS = 32
MAX_DISTANCE = 128
ROPE_BASE = 10000.0
EW = 2432             # bias table width: 1920 + 512
MASK_NEG = -30000.0


def _build_nc():
    nc = bacc.Bacc("TRN2", target_bir_lowering=False, debug=False,
                   num_devices=NC)
    f32 = mybir.dt.float32
    bf = mybir.dt.float16

    xT = nc.dram_tensor("xT", [H, NT], bf, kind="ExternalInput").ap()
    wqk = nc.dram_tensor("wqk", [H, 4 * 128], bf, kind="ExternalInput").ap()
    wv = nc.dram_tensor("wv", [H, 256], bf, kind="ExternalInput").ap()
    wd = nc.dram_tensor("wd", [H, 256], bf, kind="ExternalInput").ap()
    eb = nc.dram_tensor("eb", [128, HPC, EW], bf, kind="ExternalInput").ap()
    cosq = nc.dram_tensor("cosq", [128, S], bf, kind="ExternalInput").ap()
    s2q = nc.dram_tensor("s2q", [128, S], bf, kind="ExternalInput").ap()
    cosk = nc.dram_tensor("cosk", [128, S], bf, kind="ExternalInput").ap()
    s2k = nc.dram_tensor("s2k", [128, S], bf, kind="ExternalInput").ap()
    rotm = nc.dram_tensor("rotm", [128, 128], bf, kind="ExternalInput").ap()
    ident = nc.dram_tensor("ident", [128, 128], bf, kind="ExternalInput").ap()
    out = nc.dram_tensor("out", [NT, 256], f32, kind="ExternalOutput").ap()

    with tile.TileContext(nc) as tc:
        _kernel_body(tc, out, xT, wqk, wv, wd, eb, cosq, s2q, cosk, s2k,
                     rotm, ident)
    nc.compile()
    return nc


def _kernel_body(tc, out, xT, wqk, wv, wd, eb, cosq, s2q, cosk, s2k,
                 rotm, ident):
    nc = tc.nc
    P = 128
    f32 = mybir.dt.float32
    bf = mybir.dt.float16
    X = mybir.AxisListType.X

    xT_r = xT.rearrange("(ko p) t -> p ko t", p=P)    # [128,16,4096]
    wqk_r = wqk.rearrange("(ko p) m -> p ko m", p=P)  # [128,16,512]
    wv_r = wv.rearrange("(ko p) m -> p ko m", p=P)
    wd_r = wd.rearrange("(ko p) m -> p ko m", p=P)
    out_r = out.rearrange("(to p) m -> p to m", p=P)  # [128,32,256]

    with (
        tc.tile_pool(name="wpool", bufs=1) as wpool,
        tc.tile_pool(name="persist", bufs=1) as persist,
    ):
        wqk_sb = wpool.tile([P, KH, 512], bf)
        nc.sync.dma_start(wqk_sb[:], wqk_r)
        wv_sb = wpool.tile([P, KH, 256], bf)
        nc.sync.dma_start(wv_sb[:], wv_r)
        wd_sb = wpool.tile([P, KH, 256], bf)
        nc.sync.dma_start(wd_sb[:], wd_r)
        eb_sb = persist.tile([P, HPC, EW], bf)
        nc.sync.dma_start(eb_sb[:], eb)
        cq_sb = persist.tile([P, S], bf)
        nc.sync.dma_start(cq_sb[:], cosq)
        sq_sb = persist.tile([P, S], bf)
        nc.sync.dma_start(sq_sb[:], s2q)
        ck_sb = persist.tile([P, S], bf)
        nc.sync.dma_start(ck_sb[:], cosk)
        sk_sb = persist.tile([P, S], bf)
        nc.sync.dma_start(sk_sb[:], s2k)
        rot_sb = persist.tile([P, P], bf)
        nc.sync.dma_start(rot_sb[:], rotm)
        id_sb = persist.tile([P, P], bf)
        nc.sync.dma_start(id_sb[:], ident)

        qkT_sb = persist.tile([P, 4, NT], bf)   # m: q_h0, q_h1, k_h0, k_h1
        v_sb = persist.tile([P, 32, 256], bf)   # token-tile major, free=(2h,128)
        ctxT_sb = persist.tile([P, HPC, NT], bf)

        # ---------------- QKV projection (+ RoPE) ----------------
        CH = 512  # token chunk
        with (
            tc.tile_pool(name="xpool", bufs=2) as xpool,
            tc.tile_pool(name="qwork", bufs=3) as qwork,
            tc.tile_pool(name="psq", bufs=2, space="PSUM") as psq,
            tc.tile_pool(name="psr", bufs=2, space="PSUM") as psr,
        ):
            for cnum in range(NT // CH):
                t0 = cnum * CH
                pos = t0 % S
                x_sb = xpool.tile([P, KH, CH], bf, tag="xchunk")
                nc.sync.dma_start(x_sb[:], xT_r[:, :, t0:t0 + CH])
                for m in range(4):
                    ps = psq.tile([P, CH], f32, tag="qkacc")
                    for k in range(KH):
                        nc.tensor.matmul(ps[:], wqk_sb[:, k, m * P:(m + 1) * P],
                                         x_sb[:, k], start=(k == 0),
                                         stop=(k == KH - 1))
                    raw = qwork.tile([P, CH], bf, tag="raw")
                    nc.vector.tensor_copy(raw[:], ps[:])
                    ps2 = psr.tile([P, CH], f32, tag="rot")
                    nc.tensor.matmul(ps2[:], rot_sb[:], raw[:], start=True,
                                     stop=True)
                    c_sl = (cq_sb if m < 2 else ck_sb)[:, pos:pos + CH]
                    s_sl = (sq_sb if m < 2 else sk_sb)[:, pos:pos + CH]
                    dst = qkT_sb[:, m, t0:t0 + CH]
                    tmp = qwork.tile([P, CH], bf, tag="rtmp")
                    nc.vector.tensor_mul(dst, raw[:], c_sl)
                    nc.vector.tensor_mul(tmp[:], ps2[:], s_sl)
                    nc.vector.tensor_add(dst, dst, tmp[:])
                for tt in range(CH // P):
                    ptile = cnum * (CH // P) + tt
                    psv = psq.tile([P, 256], f32, tag="vacc")
                    for k in range(KH):
                        nc.tensor.matmul(psv[:],
                                         x_sb[:, k, tt * P:(tt + 1) * P],
                                         wv_sb[:, k], start=(k == 0),
                                         stop=(k == KH - 1))
                    nc.vector.tensor_copy(v_sb[:, ptile], psv[:])

        # ---------------- attention ----------------
        with (
            tc.tile_pool(name="awork", bufs=2) as awork,
            tc.tile_pool(name="small", bufs=3) as small,
            tc.tile_pool(name="ptbp", bufs=3) as ptbp,
            tc.tile_pool(name="pss", bufs=2, space="PSUM") as pss,
            tc.tile_pool(name="pst", bufs=3, space="PSUM") as pstp,
            tc.tile_pool(name="psc", bufs=2, space="PSUM") as pscp,
        ):
            for h in range(HPC):
                for b in range(B):
                    for qi in range(16):
                        nk = qi // 4 + 1
                        qg = b * S + qi * P
                        probs = awork.tile([P, 2048], bf, tag="probs")
                        acc = small.tile([P, 4], f32, tag="acc")
                        for ci in range(nk):
                            kg = b * S + ci * 512
                            ps = pss.tile([P, 512], f32, tag="sc")
                            nc.tensor.matmul(ps[:], qkT_sb[:, h, qg:qg + P],
                                             qkT_sb[:, 2 + h, kg:kg + 512],
                                             start=True, stop=True)
                            u0 = 1920 - qi * P + ci * 512
                            pr = probs[:, ci * 512:(ci + 1) * 512]
                            nc.vector.tensor_add(pr, ps[:],
                                                 eb_sb[:, h, u0:u0 + 512])
                            nc.scalar.activation(
                                pr, pr, mybir.ActivationFunctionType.Exp,
                                accum_out=acc[:, ci:ci + 1])
                        sumt = small.tile([P, 1], f32, tag="sumt")
                        nc.vector.reduce_sum(sumt[:], acc[:, 0:nk], axis=X)
                        rec = small.tile([P, 1], f32, tag="rec")
                        nc.vector.reciprocal(rec[:], sumt[:])
                        nc.vector.tensor_scalar_mul(probs[:, 0:nk * 512],
                                                    probs[:, 0:nk * 512],
                                                    rec[:])
                        psc = pscp.tile([P, P], f32, tag="ctx")
                        for kt in range(qi + 1):
                            pst = pstp.tile([P, P], bf, tag="tr")
                            nc.tensor.transpose(pst[:],
                                                probs[:, kt * P:(kt + 1) * P],
                                                id_sb[:])
                            ptb = ptbp.tile([P, P], bf, tag="ptb")
                            nc.vector.tensor_copy(ptb[:], pst[:])
                            nc.tensor.matmul(psc[:],
                                             v_sb[:, b * 16 + kt,
                                                  h * P:(h + 1) * P],
                                             ptb[:], start=(kt == 0),
                                             stop=(kt == qi))
                        nc.vector.tensor_copy(ctxT_sb[:, h, qg:qg + P], psc[:])

        # ---------------- AllGather ctx^T ----------------
        with tc.tile_pool(name="dram", bufs=1, space="DRAM") as dram:
            ag_in = dram.tile([HPC * P, NT], bf)
            ag_out = dram.tile([N_HEADS * P, NT], bf)
            ag_in_r = ag_in.rearrange("(hh p) t -> p hh t", p=P)
            nc.sync.dma_start(ag_in_r, ctxT_sb[:])
            nc.gpsimd.collective_compute(
                "AllGather", mybir.AluOpType.bypass,
                ins=[ag_in[:].opt()], outs=[ag_out[:].opt()],
                replica_groups=[list(range(NC))],
            )
            ag_r = ag_out.rearrange("(ko p) t -> p ko t", p=P)  # [128,16,4096]

            # ---------------- dense columns ----------------
            with (
                tc.tile_pool(name="dwork", bufs=3) as dwork,
                tc.tile_pool(name="psd", bufs=2, space="PSUM") as psd,
            ):
                for tt in range(NT // P):
                    lhs = dwork.tile([P, KH, P], bf, tag="dlhs")
                    nc.sync.dma_start(lhs[:], ag_r[:, :, tt * P:(tt + 1) * P])
                    pso = psd.tile([P, 256], f32, tag="dacc")
                    for k in range(KH):
                        nc.tensor.matmul(pso[:], lhs[:, k], wd_sb[:, k],
                                         start=(k == 0), stop=(k == KH - 1))
                    ob = dwork.tile([P, 256], f32, tag="osb")
                    nc.vector.tensor_copy(ob[:], pso[:])
                    nc.sync.dma_start(out_r[:, tt], ob[:])


def _rel_bucket_row(d):
    """T5 causal bucket for relative distance d = q - k >= 0 (scalar int)."""
    if d < NUM_BUCKETS // 2:
        return d
    me = NUM_BUCKETS // 2
    v = int(np.float32(
        np.log(np.float32(d) / me) / math.log(MAX_DISTANCE / me) * me))
    return min(me + v, NUM_BUCKETS - 1)


def _host_inputs(x, qkv_w, dense_w, rel_attn_table):
    """Build per-core input maps (all host-side layout prep, numpy only)."""
    x2 = np.ascontiguousarray(x.reshape(NT, H))
    xT = np.ascontiguousarray(x2.T).astype(BF16)

    # RoPE tables in q^T feature-major layout.
    inv = 1.0 / (ROPE_BASE ** (np.arange(0, HD, 2, dtype=np.float64) / HD))
    ang = np.arange(S, dtype=np.float64)[None, :] * inv[:, None]  # [64,S]
    cos64, sin64 = np.cos(ang), np.sin(ang)
    cosF = np.concatenate([cos64, cos64], 0)                # [128,S]
    s2F = np.concatenate([-sin64, sin64], 0)
    sc = 1.0 / math.sqrt(HD)
    cosq = (cosF * sc).astype(BF16)
    s2q = (s2F * sc).astype(BF16)
    cosk = cosF.astype(BF16)
    s2k = s2F.astype(BF16)

    rotm = np.zeros((128, 128), np.float32)
    for i in range(128):
        rotm[(i + 64) % 128, i] = 1.0
    rotm = rotm.astype(BF16)
    ident = np.eye(128, dtype=np.float32).astype(BF16)

    # rowval[d] for d in [-527, 2047]; mask folded in for d < 0.
    rowval = {}
    for h in range(N_HEADS):
        rv = np.full(EW + 128, MASK_NEG, np.float32)
        # index u in E: E[i, u] = rowval(i - u + 1920)
        rowval[h] = rv
    table = np.asarray(rel_attn_table, np.float32)
    dvals = np.arange(0, S, dtype=np.int64)
    buckets = np.array([_rel_bucket_row(int(d)) for d in dvals])

    i_idx = np.arange(128)[:, None]
    u_idx = np.arange(EW)[None, :]
    dmat = i_idx - u_idx + 1920                              # [-2431+128, 2047]

    in_maps = []
    for c in range(NC):
        h0 = HPC * c
        qcols = qkv_w[:, h0 * HD:(h0 + HPC) * HD]
        kcols = qkv_w[:, H + h0 * HD:H + (h0 + HPC) * HD]
        vcols = qkv_w[:, 2 * H + h0 * HD:2 * H + (h0 + HPC) * HD]
        wqk = np.concatenate([qcols, kcols], 1).astype(BF16)  # [H,512]
        wv = np.ascontiguousarray(vcols).astype(BF16)
        wd = np.ascontiguousarray(
            dense_w[:, c * 256:(c + 1) * 256]).astype(BF16)

        eb = np.empty((128, HPC, EW), np.float32)
        for hh in range(HPC):
            head = h0 + hh
            vals = np.where(dmat >= 0,
                            table[buckets[np.clip(dmat, 0, S - 1)], head],
                            MASK_NEG)
            eb[:, hh, :] = vals
        in_maps.append({
            "xT": xT, "wqk": wqk, "wv": wv, "wd": wd,
            "eb": eb.astype(BF16),
            "cosq": cosq, "s2q": s2q, "cosk": cosk, "s2k": s2k,
            "rotm": rotm, "ident": ident,
        })
    return in_maps


_NC_CACHE = None


def run(inputs, trace=False):
    global _NC_CACHE
    if _NC_CACHE is None:
        _NC_CACHE = _build_nc()
    in_maps = _host_inputs(**inputs)
    res = bass_utils.run_bass_kernel_spmd(
        _NC_CACHE, in_maps, core_ids=list(range(NC)), trace=trace)
    outs = [res.results[c]["out"] for c in range(NC)]
    full = np.concatenate(outs, axis=1).reshape(B, S, H).astype(np.float32)
    return full, res


def kernel(**inputs):
    full, _ = run(inputs, trace=False)
    return full


if __name__ == "__main__":
    import jax
    full, res = run({k: np.asarray(v) for k, v in
                     __import__("reference").setup_inputs().items()},
                    trace=True)
    print("exec_time_ns:", res.exec_time_ns)


# revision 6
# speedup vs baseline: 1.0262x; 1.0262x over previous
"""Trainium2 Bass kernel: 16-head attention (RoPE + T5 rel bias, causal) on 8 cores.

Sharding: tensor-parallel over heads (2 heads/core, both batch elements on
every core). Each core computes q/k/v^T for its heads from a replicated
pre-transposed x, runs causal flash attention, AllGathers ctx^T, and produces
its 256 columns of the final dense projection. Host concatenates columns.
"""

import math
import sys

sys.path.insert(0, "/opt/trn_rl_repo")

import numpy as np
import ml_dtypes

import concourse.bass as bass
import concourse.mybir as mybir
import concourse.tile as tile
from concourse import bacc
from concourse import bass_utils

BF16 = np.float16

B, S, H = 2, 2048, 2048
NT = B * S            # 4096 tokens
N_HEADS = 16
HD = 128              # head dim
NC = 8                # cores
HPC = N_HEADS // NC   # heads per core = 2
KH = H // 128         # 16 contraction tiles over hidden
NUM_BUCKETS = 32
MAX_DISTANCE = 128
ROPE_BASE = 10000.0
EW = 2432             # bias table width: 1920 + 512
MASK_NEG = -30000.0


def _build_nc():
    nc = bacc.Bacc("TRN2", target_bir_lowering=False, debug=False,
                   num_devices=NC)
    f32 = mybir.dt.float32
    bf = mybir.dt.float16

    xT = nc.dram_tensor("xT", [H, NT], bf, kind="ExternalInput").ap()
    wqk = nc.dram_tensor("wqk", [H, 4 * 128], bf, kind="ExternalInput").ap()
    wv = nc.dram_tensor("wv", [H, 256], bf, kind="ExternalInput").ap()
    wd = nc.dram_tensor("wd", [H, 256], bf, kind="ExternalInput").ap()
    eb = nc.dram_tensor("eb", [128, HPC, EW], bf, kind="ExternalInput").ap()
    cosq = nc.dram_tensor("cosq", [128, S], bf, kind="ExternalInput").ap()
    s2q = nc.dram_tensor("s2q", [128, S], bf, kind="ExternalInput").ap()
    cosk = nc.dram_tensor("cosk", [128, S], bf, kind="ExternalInput").ap()
    s2k = nc.dram_tensor("s2k", [128, S], bf, kind="ExternalInput").ap()
    rotm = nc.dram_tensor("rotm", [128, 128], bf, kind="ExternalInput").ap()
    ident = nc.dram_tensor("ident", [128, 128], bf, kind="ExternalInput").ap()
    out = nc.dram_tensor("out", [NT, 256], f32, kind="ExternalOutput").ap()

    with tile.TileContext(nc) as tc:
        _kernel_body(tc, out, xT, wqk, wv, wd, eb, cosq, s2q, cosk, s2k,
                     rotm, ident)
    nc.compile()
    return nc


def _kernel_body(tc, out, xT, wqk, wv, wd, eb, cosq, s2q, cosk, s2k,
                 rotm, ident):
    nc = tc.nc
    P = 128
    f32 = mybir.dt.float32
    bf = mybir.dt.float16
    X = mybir.AxisListType.X

    xT_r = xT.rearrange("(ko p) t -> p ko t", p=P)    # [128,16,4096]
    wqk_r = wqk.rearrange("(ko p) m -> p ko m", p=P)  # [128,16,512]
    wv_r = wv.rearrange("(ko p) m -> p ko m", p=P)
    wd_r = wd.rearrange("(ko p) m -> p ko m", p=P)
    out_r = out.rearrange("(to p) m -> p to m", p=P)  # [128,32,256]

    with (
        tc.tile_pool(name="wpool", bufs=1) as wpool,
        tc.tile_pool(name="persist", bufs=1) as persist,
    ):
        wqk_sb = wpool.tile([P, KH, 512], bf)
        nc.sync.dma_start(wqk_sb[:], wqk_r)
        wv_sb = wpool.tile([P, KH, 256], bf)
        nc.sync.dma_start(wv_sb[:], wv_r)
        wd_sb = wpool.tile([P, KH, 256], bf)
        nc.sync.dma_start(wd_sb[:], wd_r)
        eb_sb = persist.tile([P, HPC, EW], bf)
        nc.sync.dma_start(eb_sb[:], eb)
        cq_sb = persist.tile([P, S], bf)
        nc.sync.dma_start(cq_sb[:], cosq)
        sq_sb = persist.tile([P, S], bf)
        nc.sync.dma_start(sq_sb[:], s2q)
        ck_sb = persist.tile([P, S], bf)
        nc.sync.dma_start(ck_sb[:], cosk)
        sk_sb = persist.tile([P, S], bf)
        nc.sync.dma_start(sk_sb[:], s2k)
        rot_sb = persist.tile([P, P], bf)
        nc.sync.dma_start(rot_sb[:], rotm)
        id_sb = persist.tile([P, P], bf)
        nc.sync.dma_start(id_sb[:], ident)

        qkT_sb = persist.tile([P, 4, NT], bf)   # m: q_h0, q_h1, k_h0, k_h1
        v_sb = persist.tile([P, 32, 256], bf)   # token-tile major, free=(2h,128)
        ctxT_sb = persist.tile([P, HPC, NT], bf)

        # ---------------- QKV projection (+ RoPE) ----------------
        CH = 512  # token chunk
        with (
            tc.tile_pool(name="xpool", bufs=2) as xpool,
            tc.tile_pool(name="qwork", bufs=3) as qwork,
            tc.tile_pool(name="psq", bufs=3, space="PSUM") as psq,
            tc.tile_pool(name="psr", bufs=2, space="PSUM") as psr,
        ):
            for cnum in range(NT // CH):
                t0 = cnum * CH
                pos = t0 % S
                x_sb = xpool.tile([P, KH, CH], bf, tag="xchunk")
                nc.sync.dma_start(x_sb[:], xT_r[:, :, t0:t0 + CH])
                for m in range(4):
                    ps = psq.tile([P, CH], f32, tag="qkacc")
                    for k in range(KH):
                        nc.tensor.matmul(ps[:], wqk_sb[:, k, m * P:(m + 1) * P],
                                         x_sb[:, k], start=(k == 0),
                                         stop=(k == KH - 1))
                    raw = qwork.tile([P, CH], bf, tag="raw")
                    nc.vector.tensor_copy(raw[:], ps[:])
                    ps2 = psr.tile([P, CH], f32, tag="rot")
                    nc.tensor.matmul(ps2[:], rot_sb[:], raw[:], start=True,
                                     stop=True)
                    c_sl = (cq_sb if m < 2 else ck_sb)[:, pos:pos + CH]
                    s_sl = (sq_sb if m < 2 else sk_sb)[:, pos:pos + CH]
                    dst = qkT_sb[:, m, t0:t0 + CH]
                    tmp = qwork.tile([P, CH], bf, tag="rtmp")
                    nc.vector.tensor_mul(dst, raw[:], c_sl)
                    nc.vector.tensor_mul(tmp[:], ps2[:], s_sl)
                    nc.vector.tensor_add(dst, dst, tmp[:])
                for tt in range(CH // P):
                    ptile = cnum * (CH // P) + tt
                    psv = psq.tile([P, 256], f32, tag="vacc")
                    for k in range(KH):
                        nc.tensor.matmul(psv[:],
                                         x_sb[:, k, tt * P:(tt + 1) * P],
                                         wv_sb[:, k], start=(k == 0),
                                         stop=(k == KH - 1))
                    nc.vector.tensor_copy(v_sb[:, ptile], psv[:])

        # ---------------- attention ----------------
        with (
            tc.tile_pool(name="awork", bufs=2) as awork,
            tc.tile_pool(name="small", bufs=3) as small,
            tc.tile_pool(name="ptbp", bufs=3) as ptbp,
            tc.tile_pool(name="pss", bufs=3, space="PSUM") as pss,
            tc.tile_pool(name="pst", bufs=3, space="PSUM") as pstp,
            tc.tile_pool(name="psc", bufs=2, space="PSUM") as pscp,
        ):
            for b in range(B):
                for h in range(HPC):
                    for qi in range(16):
                        nk = qi // 4 + 1
                        qg = b * S + qi * P
                        probs = awork.tile([P, 2048], bf, tag="probs")
                        acc = small.tile([P, 4], f32, tag="acc")
                        for ci in range(nk):
                            kg = b * S + ci * 512
                            ps = pss.tile([P, 512], f32, tag="sc")
                            nc.tensor.matmul(ps[:], qkT_sb[:, h, qg:qg + P],
                                             qkT_sb[:, 2 + h, kg:kg + 512],
                                             start=True, stop=True)
                            u0 = 1920 - qi * P + ci * 512
                            pr = probs[:, ci * 512:(ci + 1) * 512]
                            nc.vector.tensor_add(pr, ps[:],
                                                 eb_sb[:, h, u0:u0 + 512])
                            nc.scalar.activation(
                                pr, pr, mybir.ActivationFunctionType.Exp,
                                accum_out=acc[:, ci:ci + 1])
                        sumt = small.tile([P, 1], f32, tag="sumt")
                        nc.vector.reduce_sum(sumt[:], acc[:, 0:nk], axis=X)
                        rec = small.tile([P, 1], f32, tag="rec")
                        nc.vector.reciprocal(rec[:], sumt[:])
                        nc.vector.tensor_scalar_mul(probs[:, 0:nk * 512],
                                                    probs[:, 0:nk * 512],
                                                    rec[:])
                        psc = pscp.tile([P, P], f32, tag="ctx")
                        for kt in range(qi + 1):
                            pst = pstp.tile([P, P], bf, tag="tr")
                            nc.tensor.transpose(pst[:],
                                                probs[:, kt * P:(kt + 1) * P],
                                                id_sb[:])
                            ptb = ptbp.tile([P, P], bf, tag="ptb")
                            nc.vector.tensor_copy(ptb[:], pst[:])
                            nc.tensor.matmul(psc[:],
                                             v_sb[:, b * 16 + kt,
                                                  h * P:(h + 1) * P],
                                             ptb[:], start=(kt == 0),
                                             stop=(kt == qi))
                        nc.vector.tensor_copy(ctxT_sb[:, h, qg:qg + P], psc[:])

        # ---- AllGather ctx^T + dense, pipelined per token-half (batch) ----
        # Attention runs batch-0 first, so half 0's AllGather + dense overlap
        # batch-1 attention.
        with (
            tc.tile_pool(name="dram", bufs=1, space="DRAM") as dram,
            tc.tile_pool(name="dwork", bufs=3) as dwork,
            tc.tile_pool(name="psd", bufs=2, space="PSUM") as psd,
        ):
            for half in range(B):
                hb = half * S
                ag_in = dram.tile([HPC * P, S], bf, tag=f"agi{half}")
                ag_out = dram.tile([N_HEADS * P, S], bf, tag=f"ago{half}",
                                   addr_space="Shared")
                ag_in_r = ag_in.rearrange("(hh p) t -> p hh t", p=P)
                nc.sync.dma_start(ag_in_r, ctxT_sb[:, :, hb:hb + S])
                nc.gpsimd.collective_compute(
                    "AllGather", mybir.AluOpType.bypass,
                    ins=[ag_in[:].opt()], outs=[ag_out[:].opt()],
                    replica_groups=[list(range(NC))],
                )
                ag_r = ag_out.rearrange("(ko p) t -> p ko t", p=P)
                for tt in range(S // P):
                    lhs = dwork.tile([P, KH, P], bf, tag="dlhs")
                    nc.sync.dma_start(lhs[:], ag_r[:, :, tt * P:(tt + 1) * P])
                    pso = psd.tile([P, 256], f32, tag="dacc")
                    for k in range(KH):
                        nc.tensor.matmul(pso[:], lhs[:, k], wd_sb[:, k],
                                         start=(k == 0), stop=(k == KH - 1))
                    ob = dwork.tile([P, 256], f32, tag="osb")
                    nc.vector.tensor_copy(ob[:], pso[:])
                    nc.sync.dma_start(out_r[:, half * 16 + tt], ob[:])


# Exact T5 bucket boundaries matching the reference's float32 bucketing:
# bucket(d) = searchsorted(_BUCKET_STARTS, d, 'right') - 1 for d >= 0.
_BUCKET_STARTS = np.array(
    [0, 1, 2, 3, 4, 5, 6, 7, 8, 9, 10, 11, 12, 13, 14, 15,
     16, 18, 20, 23, 26, 29, 33, 38, 43, 49, 55, 63, 72, 82, 93, 106],
    dtype=np.int64)


def _rel_bucket_row(d):
    """T5 causal bucket for relative distance d = q - k >= 0 (scalar int)."""
    return int(np.searchsorted(_BUCKET_STARTS, d, side="right") - 1)


def _host_inputs(x, qkv_w, dense_w, rel_attn_table):
    """Build per-core input maps (all host-side layout prep, numpy only)."""
    x2 = np.ascontiguousarray(x.reshape(NT, H))
    xT = np.ascontiguousarray(x2.T).astype(BF16)

    # RoPE tables in q^T feature-major layout.
    inv = 1.0 / (ROPE_BASE ** (np.arange(0, HD, 2, dtype=np.float64) / HD))
    ang = np.arange(S, dtype=np.float64)[None, :] * inv[:, None]  # [64,S]
    cos64, sin64 = np.cos(ang), np.sin(ang)
    cosF = np.concatenate([cos64, cos64], 0)                # [128,S]
    s2F = np.concatenate([-sin64, sin64], 0)
    sc = 1.0 / math.sqrt(HD)
    cosq = (cosF * sc).astype(BF16)
    s2q = (s2F * sc).astype(BF16)
    cosk = cosF.astype(BF16)
    s2k = s2F.astype(BF16)

    rotm = np.zeros((128, 128), np.float32)
    for i in range(128):
        rotm[(i + 64) % 128, i] = 1.0
    rotm = rotm.astype(BF16)
    ident = np.eye(128, dtype=np.float32).astype(BF16)

    # rowval[d] for d in [-527, 2047]; mask folded in for d < 0.
    rowval = {}
    for h in range(N_HEADS):
        rv = np.full(EW + 128, MASK_NEG, np.float32)
        # index u in E: E[i, u] = rowval(i - u + 1920)
        rowval[h] = rv
    table = np.asarray(rel_attn_table, np.float32)
    dvals = np.arange(0, S, dtype=np.int64)
    buckets = np.array([_rel_bucket_row(int(d)) for d in dvals])

    i_idx = np.arange(128)[:, None]
    u_idx = np.arange(EW)[None, :]
    dmat = i_idx - u_idx + 1920                              # [-2431+128, 2047]

    in_maps = []
    for c in range(NC):
        h0 = HPC * c
        qcols = qkv_w[:, h0 * HD:(h0 + HPC) * HD]
        kcols = qkv_w[:, H + h0 * HD:H + (h0 + HPC) * HD]
        vcols = qkv_w[:, 2 * H + h0 * HD:2 * H + (h0 + HPC) * HD]
        wqk = np.concatenate([qcols, kcols], 1).astype(BF16)  # [H,512]
        wv = np.ascontiguousarray(vcols).astype(BF16)
        wd = np.ascontiguousarray(
            dense_w[:, c * 256:(c + 1) * 256]).astype(BF16)

        eb = np.empty((128, HPC, EW), np.float32)
        for hh in range(HPC):
            head = h0 + hh
            vals = np.where(dmat >= 0,
                            table[buckets[np.clip(dmat, 0, S - 1)], head],
                            MASK_NEG)
            eb[:, hh, :] = vals
        in_maps.append({
            "xT": xT, "wqk": wqk, "wv": wv, "wd": wd,
            "eb": eb.astype(BF16),
            "cosq": cosq, "s2q": s2q, "cosk": cosk, "s2k": s2k,
            "rotm": rotm, "ident": ident,
        })
    return in_maps


_NC_CACHE = None


def run(inputs, trace=False):
    global _NC_CACHE
    if _NC_CACHE is None:
        _NC_CACHE = _build_nc()
    in_maps = _host_inputs(**inputs)
    res = bass_utils.run_bass_kernel_spmd(
        _NC_CACHE, in_maps, core_ids=list(range(NC)), trace=trace)
    outs = [res.results[c]["out"] for c in range(NC)]
    full = np.concatenate(outs, axis=1).reshape(B, S, H).astype(np.float32)
    return full, res


def kernel(**inputs):
    full, _ = run(inputs, trace=False)
    return full


if __name__ == "__main__":
    import jax
    full, res = run({k: np.asarray(v) for k, v in
                     __import__("reference").setup_inputs().items()},
                    trace=True)
    print("exec_time_ns:", res.exec_time_ns)
